# revision 1
# baseline (speedup 1.0000x reference)
# Causal self-attention (B=4, T=2048, C=1024, 16 heads) on 8 NeuronCores.
#
# Sharding: core i = (batch b = i//2, head-group g = i%2).  Each core runs the
# full attention pipeline for one batch element and 8 of the 16 heads:
#   qT,kT = Wqk^T @ x^T       (cols-on-partitions layout, bias on DVE eviction)
#   v     = x @ Wv + bv       (token-on-partitions; a ones-column is prepended
#                              per head, bias broadcast via GpSimd)
#   S^T   = kT-tiles^T @ qT   (keys on partitions; head pairs packed onto
#                              disjoint PE row groups; causal column trim)
#   P     = exp(S/8) * causal-mask            (ACT exp straight from PSUM)
#   yT_aug= v_aug^T @ P       (row 0 of each head's block = softmax denom)
#   yT    = yT_aug[1:65] * bcast(1/denom)     (GpSimd partition_broadcast)
#   out  += yT^T @ Wproj_g    (partial over head-group; summed on host)
# Host adds the two per-batch partials + b_proj.
import numpy as np
import ml_dtypes

import concourse.tile as tile
from concourse import bacc, mybir
from concourse.bass_utils import run_bass_kernel_spmd

BF16 = mybir.dt.bfloat16
F32 = mybir.dt.float32
AF = mybir.ActivationFunctionType
MULT = mybir.AluOpType.mult

# Full-problem constants (hardcoded; kernel.py must be self-contained).
B, T, C, N_HEAD = 4, 2048, 1024, 16
D = C // N_HEAD            # 64
H = N_HEAD // 2            # 8 heads per core
GC = H * D                 # 512 group cols
P = 128


def build_nc(T=T, C=C, H=H, D=D, trace=False):
    """Build the single-core Bass/Tile program (shared SPMD across 8 cores)."""
    KC = C // P                 # contraction chunks for C
    NT = T // P                 # token tiles
    TQ = min(512, T)            # query-chunk width
    NJ = T // TQ                # query chunks
    NM = TQ // P                # diagonal score tiles per query chunk
    GC_ = H * D
    GKC = GC_ // P              # contraction chunks for group cols
    VW = D + 1                  # per-head v width incl. ones column

    nc = bacc.Bacc("TRN2", target_bir_lowering=False, debug=False)

    xT_d = nc.dram_tensor("xT", [C, T], BF16, kind="ExternalInput")
    wqk_d = nc.dram_tensor("wqk", [C, 2 * GC_], BF16, kind="ExternalInput")
    bqk_d = nc.dram_tensor("bqk", [2 * GC_], F32, kind="ExternalInput")
    wv_d = nc.dram_tensor("wv", [C, GC_], BF16, kind="ExternalInput")
    bv_d = nc.dram_tensor("bv", [GC_], F32, kind="ExternalInput")
    wp_d = nc.dram_tensor("wp", [GC_, C], BF16, kind="ExternalInput")
    masks_d = nc.dram_tensor("masks", [NM, P, TQ], BF16, kind="ExternalInput")
    out_d = nc.dram_tensor("out", [T, C], F32, kind="ExternalOutput")

    with tile.TileContext(nc) as tc:
        with (
            tc.tile_pool(name="persist", bufs=1) as persist,
            tc.tile_pool(name="ptile", bufs=6) as ptile,
            tc.tile_pool(name="evict", bufs=4) as evict,
            tc.tile_pool(name="mm_psum", bufs=2, space="PSUM") as mm_psum,
            tc.tile_pool(name="s_psum", bufs=2, space="PSUM") as s_psum,
            tc.tile_pool(name="y_psum", bufs=1, space="PSUM") as y_psum,
        ):
            # ---- persistent SBUF tensors + loads.  Order matters: the v
            # phase runs first, so its inputs (x column chunk 0, wv) go out
            # first; wqk streams per column-chunk in pair order during it.
            wv_sb = persist.tile([P, KC, GC_], BF16)
            wv_r = wv_d.ap().rearrange("(kc p) m -> p kc m", p=P)
            xT_sb = persist.tile([P, KC, T], BF16)
            xT_r = xT_d.ap().rearrange("(kc p) t -> p kc t", p=P)
            wqk_sb = persist.tile([P, KC, 2 * GC_], BF16)
            wqk_r = wqk_d.ap().rearrange("(kc p) m -> p kc m", p=P)

            def load_wqk(c):
                for m in (c, GKC + c):  # pair order: q chunk then k chunk
                    ms = slice(m * P, (m + 1) * P)
                    nc.sync.dma_start(wqk_sb[:, :, ms], wqk_r[:, :, ms])

            # load order follows first-use order: wqk pair 0 + x chunk 0
            # (first qkT matmuls), bias, remaining x chunks, v weights, masks
            load_wqk(0)
            nc.sync.dma_start(xT_sb[:, :KC // 2, :TQ], xT_r[:, :KC // 2, :TQ])
            nc.sync.dma_start(xT_sb[:, KC // 2:, :TQ], xT_r[:, KC // 2:, :TQ])
            bqk_sb = persist.tile([P, 2 * GKC], F32)
            nc.sync.dma_start(bqk_sb[:], bqk_d.ap().rearrange("(kc p) -> p kc", p=P))
            # wv + masks before the xT j1.. chunks: the first strip's exp/
            # mask/AV chain unblocks ~30us earlier, keeping the ACT engine
            # (the real-HW phase-3 bottleneck) continuously fed from ~14us
            nc.sync.dma_start(wv_sb[:, :KC // 2], wv_r[:, :KC // 2])
            nc.sync.dma_start(wv_sb[:, KC // 2:], wv_r[:, KC // 2:])
            masks_sb = persist.tile([P, NM, TQ], BF16)
            nc.sync.dma_start(masks_sb[:], masks_d.ap().rearrange("m p f -> p m f"))
            bv_sb = persist.tile([1, GC_], F32)
            nc.sync.dma_start(bv_sb[:], bv_d.ap()[None, :])
            bvb = persist.tile([P, GC_], F32)
            nc.gpsimd.partition_broadcast(bvb[:], bv_sb[:])
            for j in range(1, NJ):
                js = slice(j * TQ, (j + 1) * TQ)
                nc.sync.dma_start(xT_sb[:, :, js], xT_r[:, :, js])
            for c in range(1, GKC):
                load_wqk(c)

            # DVE "touch": absorb DMA waits into the DVE vector clock before
            # their first 2-input consumers.
            scrap = persist.tile([P, 2], F32)
            nc.vector.tensor_copy(scrap[:, 0:1], bqk_sb[:, 0:1])
            nc.vector.tensor_copy(scrap[:, 1:2], masks_sb[:, 0, 0:1])

            qkT_sb = persist.tile([P, 2 * GKC, T], BF16)
            v_sb = persist.tile([P, NT, H * VW], BF16)
            nc.vector.memset(v_sb[:], 1.0)
            yT_sb = persist.tile([P, GKC, T], BF16)

            # ---- v = x @ Wv, bias added on eviction (ones col is pre-memset
            # col 0 of each head's VW block).  Emitted in tile ranges,
            # interleaved into pair 0's strips so attention starts early.
            def v_tiles(trange):
                for t in trange:
                    ps = mm_psum.tile([P, GC_], F32, tag="mm")
                    for kc in range(KC):
                        nc.tensor.matmul(
                            ps[:],
                            xT_sb[:, kc, t * P:(t + 1) * P],
                            wv_sb[:, kc, :],
                            start=(kc == 0), stop=(kc == KC - 1),
                        )
                    nc.vector.tensor_tensor(
                        v_sb[:, t].rearrange("p (h e) -> p h e", h=H)[:, :, 1:],
                        ps[:].rearrange("p (h e) -> p h e", h=H),
                        bvb[:].rearrange("p (h e) -> p h e", h=H),
                        mybir.AluOpType.add,
                    )

            wp_sb = persist.tile([P, GKC, C], BF16)
            nc.sync.dma_start(wp_sb[:], wp_d.ap().rearrange("(kc p) m -> p kc m", p=P))

            # ---- phase 2+3 pipelined per head pair: qT,kT for pair c
            # (qkT = Wqk^T @ x^T, bias on DVE eviction), then attention for
            # pair c.  The scheduler overlaps pair c+1's qkT matmuls with the
            # ACT-bound exp stream of pair c's attention.
            def qkT_groups(c, js):
                # j outer: halves the early demand rate on the xT DMA stream
                for j in js:
                    for m in (c, GKC + c):
                        ps = mm_psum.tile([P, TQ], F32, tag="mm")
                        for kc in range(KC):
                            nc.tensor.matmul(
                                ps[:],
                                wqk_sb[:, kc, m * P:(m + 1) * P],
                                xT_sb[:, kc, j * TQ:(j + 1) * TQ],
                                start=(kc == 0), stop=(kc == KC - 1),
                            )
                        nc.vector.tensor_tensor(
                            qkT_sb[:, m, j * TQ:(j + 1) * TQ], ps[:],
                            bqk_sb[:, m:m + 1].to_broadcast((P, TQ)),
                            mybir.AluOpType.add,
                        )

            # ---- phase 3: attention, head pairs interleaved so the two
            # K=64 score matmuls occupy disjoint PE row groups (rows 0-63 /
            # 64-127) and run concurrently.  Diagonal tiles are column-
            # trimmed: for tk-tile i = j*NM + m, query columns < 128*m are
            # fully masked, so scores/exp/mask/AV skip them.
            scale = float(1.0 / np.sqrt(D))

            def norm_evict(y_ps, h, j):
                c, qp = h // 2, (h % 2) * D
                rc = evict.tile([1, TQ], F32, tag=f"rc{h % 2}")
                # ~18-bit reciprocal, 5x faster than exact; denom in [1, 2e3]
                nc.vector.reciprocal_approx_fast(rc[:], y_ps[0:1, :])
                bc = evict.tile([P, TQ], F32, tag=f"bc{h % 2}")
                nc.gpsimd.partition_broadcast(bc[:D + 1, :], rc[:])
                tmp = evict.tile([P, TQ], BF16, tag=f"ytmp{h % 2}")
                # DVE needs 32-aligned start partition: compute rows 0..64
                # (row 0 = denom*recip, discarded), DMA-shift rows 1..64.
                nc.vector.tensor_tensor(
                    tmp[:D + 1, :], y_ps[:D + 1, :], bc[:D + 1, :], MULT)
                nc.sync.dma_start(
                    yT_sb[qp:qp + D, c, j * TQ:(j + 1) * TQ], tmp[1:D + 1, :])

            for c in range(GKC):
                qkT_groups(c, range(NJ))
                hA, hB = 2 * c, 2 * c + 1
                if c == 0:
                    # v tiles for strips 0 and 1 up front, then two strips
                    # ahead of use so AV ldweights never waits on the evict
                    v_tiles(range(0, min(2 * NM, NT)))
                for j in range(NJ):
                    if c == 0 and j + 2 < NJ:
                        v_tiles(range(NM * (j + 2), NM * (j + 3)))
                    yA = y_psum.tile([VW, TQ], F32, tag="yA")
                    yB = y_psum.tile([VW, TQ], F32, tag="yB")
                    ilast = (j + 1) * NM - 1
                    for i in range(ilast + 1):
                        m = i - j * NM
                        lo = P * m if m > 0 else 0
                        cs = slice(j * TQ + lo, (j + 1) * TQ)
                        ls = slice(lo, TQ)
                        # one 2-bank psum tile holds both heads' score tiles;
                        # exp and mask then run as single double-width ops
                        st = s_psum.tile([P, 2, TQ], F32, tag="s")
                        kt = slice(i * P, (i + 1) * P)
                        nc.tensor.matmul(st[:, 0, ls], qkT_sb[:D, GKC + c, kt],
                                         qkT_sb[:D, c, cs], start=True, stop=True)
                        nc.tensor.matmul(st[:, 1, ls], qkT_sb[D:, GKC + c, kt],
                                         qkT_sb[D:, c, cs], start=True, stop=True)
                        pt = ptile.tile([P, 2, TQ], BF16, tag="p")
                        nc.scalar.activation(pt[:, :, ls], st[:, :, ls],
                                             AF.Exp, scale=scale)
                        if m >= 0:  # diagonal: causal mask (same for A and B)
                            # high priority: the AV matmuls stall if this TT
                            # queues behind other DVE work
                            with tc.high_priority():
                                nc.vector.tensor_tensor(
                                    pt[:, :, ls], pt[:, :, ls],
                                    masks_sb[:, m, None, ls].to_broadcast(
                                        (P, 2, TQ - lo)),
                                    MULT)
                        nc.tensor.matmul(yA[:, ls], v_sb[:, i, hA * VW:(hA + 1) * VW],
                                         pt[:, 0, ls], start=(i == 0), stop=(i == ilast))
                        nc.tensor.matmul(yB[:, ls], v_sb[:, i, hB * VW:(hB + 1) * VW],
                                         pt[:, 1, ls], start=(i == 0), stop=(i == ilast))
                    norm_evict(yA, hA, j)
                    norm_evict(yB, hB, j)

            # ---- phase 4: out_partial = yT^T @ Wproj
            for t in range(NT):
                ot = evict.tile([P, C], F32, tag="out")
                for nn in range(C // TQ):
                    ps = mm_psum.tile([P, TQ], F32, tag="mm")
                    for kc in range(GKC):
                        nc.tensor.matmul(
                            ps[:],
                            yT_sb[:, kc, t * P:(t + 1) * P],
                            wp_sb[:, kc, nn * TQ:(nn + 1) * TQ],
                            start=(kc == 0), stop=(kc == GKC - 1),
                        )
                    # ACT is idle once the exp stream ends; keep DVE free for
                    # the tail normalize chains
                    nc.scalar.activation(ot[:, nn * TQ:(nn + 1) * TQ], ps[:],
                                         AF.Copy)
                nc.sync.dma_start(out_d.ap()[t * P:(t + 1) * P, :], ot[:])

    # Bacc's compile pipeline splits multi-sem waits into event/nop
    # instructions (the 64B ISA slots hold only one wait), auto-inserts
    # gpsimd library loads and ACT table loads, and lowers extended insts.
    nc.compile()
    return nc


def make_masks(TQ=512, NM=4):
    f = np.arange(TQ)[None, :]
    p = np.arange(P)[:, None]
    m = np.stack([(f >= (P * k + p)) for k in range(NM)])
    return m.astype(ml_dtypes.bfloat16)


def make_in_maps(x, W_attn, b_attn, W_proj):
    bf16 = ml_dtypes.bfloat16
    masks = make_masks(min(512, T), min(512, T) // P)
    xTs = [np.ascontiguousarray(np.asarray(x[b]).T).astype(bf16)
           for b in range(B)]
    per_g = []
    for g in range(2):
        s = slice(g * GC, (g + 1) * GC)
        per_g.append({
            "wqk": np.ascontiguousarray(np.concatenate(
                [W_attn[:, s], W_attn[:, C:][:, s]], axis=1)).astype(bf16),
            "bqk": np.concatenate([b_attn[s], b_attn[C:][s]]).astype(np.float32),
            "wv": np.ascontiguousarray(W_attn[:, 2 * C:][:, s]).astype(bf16),
            "bv": b_attn[2 * C:][s].astype(np.float32),
            "wp": np.ascontiguousarray(W_proj[s, :]).astype(bf16),
            "masks": masks,
        })
    return [{"xT": xTs[core // 2], **per_g[core % 2]} for core in range(8)]


_NC_CACHE = {}


def kernel(x, W_attn, b_attn, W_proj, b_proj):
    x = np.asarray(x)
    W_attn = np.asarray(W_attn)
    b_attn = np.asarray(b_attn)
    W_proj = np.asarray(W_proj)
    b_proj = np.asarray(b_proj)

    if "nc" not in _NC_CACHE:
        _NC_CACHE["nc"] = build_nc()
    nc = _NC_CACHE["nc"]
    in_maps = make_in_maps(x, W_attn, b_attn, W_proj)
    try:
        res = run_bass_kernel_spmd(nc, in_maps, list(range(8)), trace=False)
    except Exception:
        # transient NRT_EXEC_UNIT_UNRECOVERABLE device wedges have been
        # observed on this fleet; one retry usually clears them
        import time as _time
        _time.sleep(5)
        res = run_bass_kernel_spmd(nc, in_maps, list(range(8)), trace=False)
    out = np.empty((B, T, C), np.float32)
    for b in range(B):
        out[b] = res.results[2 * b]["out"] + res.results[2 * b + 1]["out"] \
            + b_proj[None, :]
    return out



# revision 3
# speedup vs baseline: 1.0488x; 1.0488x over previous
# Causal self-attention (B=4, T=2048, C=1024, 16 heads) on 8 NeuronCores.
#
# Sharding: core i = (batch b = i//2, head-group g = i%2).  Each core runs the
# full attention pipeline for one batch element and 8 of the 16 heads:
#   qT,kT = Wqk^T @ x^T     fp8e4 DoubleRow matmuls (paired K chunks), bias
#                           added on the DVE eviction which quantizes to fp8
#   v     = x @ Wv + bv     bf16 (fp8 on the v path costs too much accuracy)
#   S^T   = kT^T @ qT       fp8e4 DoubleRow with a zero-padded second slot
#                           (K=64; slot1 of k is zeros, q slot dim stride-0)
#   P     = exp(S/8) * causal-mask          (ACT exp straight from PSUM, bf16)
#   y_q   = P^T-chunks @ v_aug              (stationary P [128k,128q], moving
#           v [128k,65] -> out [128q,65]: full-width output partitions; the
#           4 query-subtile accumulation groups share one PSUM bank via a
#           single-start lazy-zero pattern)
#   y     = y_q[:,1:65] * recip(y_q[:,0])   (denominator from v ones column)
#   yT    = PE-transpose(y)                 (query-major -> head-dim-major)
#   out  += yT^T @ Wproj_g  (partial over head-group; summed on host)
# Host adds the two per-batch partials + b_proj.
import numpy as np
import ml_dtypes

import concourse.tile as tile
from concourse import bacc, mybir
from concourse.bass_utils import run_bass_kernel_spmd

BF16 = mybir.dt.bfloat16
F8 = mybir.dt.float8e4
F32 = mybir.dt.float32
AF = mybir.ActivationFunctionType
MULT = mybir.AluOpType.mult
ADD = mybir.AluOpType.add
DR = mybir.MatmulPerfMode.DoubleRow

# Full-problem constants (hardcoded; kernel.py must be self-contained).
B, T, C, N_HEAD = 4, 2048, 1024, 16
D = C // N_HEAD            # 64
H = N_HEAD // 2            # 8 heads per core
GC = H * D                 # 512 group cols
P = 128


def build_nc(T=T, C=C, H=H, D=D, trace=False):
    """Build the single-core Bass/Tile program (shared SPMD across 8 cores)."""
    KC = C // P                 # contraction chunks for C
    NT = T // P                 # token tiles
    TQ = min(512, T)            # query-chunk width
    NJ = T // TQ                # query chunks (windows)
    NM = TQ // P                # query subtiles per chunk
    GC_ = H * D
    GKC = GC_ // P              # head pairs / contraction chunks for group cols
    VW = D + 1                  # per-head v width incl. ones column

    nc = bacc.Bacc("TRN2", target_bir_lowering=False, debug=False)

    xT_d = nc.dram_tensor("xT", [C, T], BF16, kind="ExternalInput")
    x8_d = nc.dram_tensor("x8", [C, T], F8, kind="ExternalInput")
    wqk_d = nc.dram_tensor("wqk", [C, 2 * GC_], F8, kind="ExternalInput")
    bqk_d = nc.dram_tensor("bqk", [2 * GC_], F32, kind="ExternalInput")
    wv_d = nc.dram_tensor("wv", [C, GC_], BF16, kind="ExternalInput")
    bv_d = nc.dram_tensor("bv", [GC_], F32, kind="ExternalInput")
    wp_d = nc.dram_tensor("wp", [GC_, C], BF16, kind="ExternalInput")
    masks_d = nc.dram_tensor("masks", [NM, P, TQ], BF16, kind="ExternalInput")
    idn_d = nc.dram_tensor("idn", [P, P], BF16, kind="ExternalInput")
    out_d = nc.dram_tensor("out", [T, C], F32, kind="ExternalOutput")

    scale = float(1.0 / np.sqrt(D))

    with tile.TileContext(nc) as tc:
        with (
            tc.tile_pool(name="persist", bufs=1) as persist,
            tc.tile_pool(name="ptile", bufs=14) as ptile,
            tc.tile_pool(name="evict", bufs=2) as evict,
            tc.tile_pool(name="s_psum", bufs=2, space="PSUM") as s_psum,
            tc.tile_pool(name="y_psum", bufs=1, space="PSUM") as y_psum,
            tc.tile_pool(name="mm_psum", bufs=2, space="PSUM") as mm_psum,
        ):
            # ---- persistent SBUF tensors.  DMA order = first-use order.
            bqk_sb = persist.tile([P, 2 * GKC], F32)
            nc.sync.dma_start(bqk_sb[:], bqk_d.ap().rearrange("(m p) -> p m", p=P))
            bv_sb = persist.tile([1, GC_], F32)
            nc.sync.dma_start(bv_sb[:], bv_d.ap()[None, :])
            idn_sb = persist.tile([P, P], BF16)
            nc.sync.dma_start(idn_sb[:], idn_d.ap())

            wqk_sb = persist.tile([P, KC, 2 * GC_], F8)
            wqk_r = wqk_d.ap().rearrange("(kc p) m -> p kc m", p=P)
            nc.sync.dma_start(wqk_sb[:], wqk_r)

            x8_sb = persist.tile([P, KC, T], F8)
            x8_r = x8_d.ap().rearrange("(kc p) t -> p kc t", p=P)
            nc.sync.dma_start(x8_sb[:, :, 0:TQ], x8_r[:, :, 0:TQ])

            masks_sb = persist.tile([P, NM, TQ], BF16)
            nc.sync.dma_start(masks_sb[:], masks_d.ap().rearrange("m p f -> p m f"))

            wv_sb = persist.tile([P, KC, GC_], BF16)
            nc.sync.dma_start(wv_sb[:], wv_d.ap().rearrange("(kc p) m -> p kc m", p=P))
            xT_sb = persist.tile([P, KC, T], BF16)
            xT_r = xT_d.ap().rearrange("(kc p) t -> p kc t", p=P)
            for t in range(NM):  # first window's token tiles, fine-grained
                ts_ = slice(t * P, (t + 1) * P)
                nc.sync.dma_start(xT_sb[:, :, ts_], xT_r[:, :, ts_])

            # remaining windows' x8/xT, then wp (first used ~1.5 windows in)
            for j in range(1, NJ):
                js = slice(j * TQ, (j + 1) * TQ)
                nc.sync.dma_start(x8_sb[:, :, js], x8_r[:, :, js])
                nc.sync.dma_start(xT_sb[:, :, js], xT_r[:, :, js])
            wp_sb = persist.tile([P, GKC, C], BF16)
            nc.sync.dma_start(wp_sb[:], wp_d.ap().rearrange("(kc p) m -> p kc m", p=P))

            # gpsimd: bias broadcast for v eviction + zero slots for the
            # score DoubleRow stationary (slot 2 of qk8 per pair)
            bvb = persist.tile([P, GC_], F32)
            nc.gpsimd.partition_broadcast(bvb[:], bv_sb[:])

            # qk8 layout: [p, pair, {0:q, 1:k, 2:zeros}, T].  Head A of pair c
            # lives on partitions 0-63, head B on 64-127 (d on partitions).
            qk8_sb = persist.tile([P, GKC, 3, T], F8)
            for c in range(GKC):
                nc.gpsimd.memset(qk8_sb[:, c, 2, :], 0.0)

            # DVE "touch": absorb DMA waits into the DVE vector clock before
            # their first 2-input consumers.
            scrap = persist.tile([P, 2], F32)
            nc.vector.tensor_copy(scrap[:, 0:1], bqk_sb[:, 0:1])
            nc.vector.tensor_copy(scrap[:, 1:2], masks_sb[:, 0, 0:1])

            v_sb = persist.tile([P, NT, H, VW], BF16)
            nc.vector.memset(v_sb[:, :, :, 0:1], 1.0)     # ones columns only
            y_sb = persist.tile([P, NT, GKC, 2, D], BF16)  # partition = query
            yT_sb = persist.tile([P, GKC, T], BF16)        # partition = (a,d)

            # ---- phase helpers -------------------------------------------
            def v_tile(t):
                """v = x @ Wv + bv for one 128-token tile (bf16)."""
                ps = mm_psum.tile([P, TQ], F32, tag="mm")
                for kc in range(KC):
                    nc.tensor.matmul(
                        ps[:],
                        xT_sb[:, kc, t * P:(t + 1) * P],
                        wv_sb[:, kc, :],
                        start=(kc == 0), stop=(kc == KC - 1),
                    )
                nc.vector.tensor_tensor(
                    v_sb[:, t, :, 1:],
                    ps[:].rearrange("p (h e) -> p h e", h=H),
                    bvb[:].rearrange("p (h e) -> p h e", h=H),
                    ADD,
                )

            def qk_proj(c, j):
                """q,k chunks for pair c, query window j: fp8 DoubleRow over
                paired K chunks; eviction adds bias and quantizes to fp8."""
                js = slice(j * TQ, (j + 1) * TQ)
                for qk, m in ((0, c), (1, GKC + c)):
                    ps = mm_psum.tile([P, TQ], F32, tag="mm")
                    for t2 in range(KC // 2):
                        nc.tensor.matmul(
                            ps[:],
                            wqk_sb[:, 2 * t2:2 * t2 + 2, m * P:(m + 1) * P],
                            x8_sb[:, 2 * t2:2 * t2 + 2, js],
                            start=(t2 == 0), stop=(t2 == KC // 2 - 1),
                            perf_mode=DR,
                        )
                    nc.vector.tensor_tensor(
                        qk8_sb[:, c, qk, js], ps[:],
                        bqk_sb[:, m:m + 1].to_broadcast((P, TQ)),
                        ADD,
                    )

            def out_proj(t):
                """out partial = yT^T @ Wproj for one token tile (bf16)."""
                ot = evict.tile([P, C], F32, tag="out", bufs=3)
                for nn in range(C // TQ):
                    ps = mm_psum.tile([P, TQ], F32, tag="mm")
                    for kc in range(GKC):
                        nc.tensor.matmul(
                            ps[:],
                            yT_sb[:, kc, t * P:(t + 1) * P],
                            wp_sb[:, kc, nn * TQ:(nn + 1) * TQ],
                            start=(kc == 0), stop=(kc == GKC - 1),
                        )
                    nc.vector.tensor_copy(ot[:, nn * TQ:(nn + 1) * TQ], ps[:])
                nc.sync.dma_start(out_d.ap()[t * P:(t + 1) * P, :], ot[:])

            def transpose_tile(c, qt):
                """y_sb [128q, (a d)] -> yT_sb [(a d), 128q] via PE transpose
                through a borrowed mm-ring PSUM slot."""
                tp = mm_psum.tile([P, P], BF16, tag="mm")
                nc.tensor.transpose(tp[:], y_sb[:, qt, c], idn_sb[:])
                nc.vector.tensor_copy(
                    yT_sb[:, c, qt * P:(qt + 1) * P], tp[:])

            def attention(c, j):
                """Scores+exp+mask+AV+normalize for pair c, query window j."""
                hA, hB = 2 * c, 2 * c + 1
                yt = {}
                for a, tag in ((0, "yA"), (1, "yB")):
                    yt[a] = y_psum.tile([P, NM, VW], F32, tag=tag, name=tag)
                ilast = 4 * j + NM - 1
                for i in range(ilast + 1):
                    m = i - 4 * j
                    lo = P * m if m > 0 else 0
                    ls = slice(lo, TQ)
                    cs = slice(j * TQ + lo, (j + 1) * TQ)
                    kt = slice(i * P, (i + 1) * P)
                    st = s_psum.tile([P, 2, TQ], F32, tag="s")
                    for a in (0, 1):
                        pb = slice(64 * a, 64 * a + 64)
                        nc.tensor.matmul(
                            st[:, a, ls],
                            qk8_sb[pb, c, 1:3, kt],
                            qk8_sb[pb, c, 0:1, cs].to_broadcast(
                                (64, 2, TQ - lo)),
                            start=True, stop=True, perf_mode=DR,
                        )
                    pt = ptile.tile([P, 2, TQ], BF16, tag="p")
                    nc.scalar.activation(pt[:, :, ls], st[:, :, ls],
                                         AF.Exp, scale=scale)
                    if m >= 0:  # diagonal: causal mask (same for A and B)
                        with tc.high_priority():
                            nc.vector.tensor_tensor(
                                pt[:, :, ls], pt[:, :, ls],
                                masks_sb[:, m, None, ls].to_broadcast(
                                    (P, 2, TQ - lo)),
                                MULT)
                    q0 = max(m, 0)
                    for a, h in ((0, hA), (1, hB)):
                        for qq in range(q0, NM):
                            nc.tensor.matmul(
                                yt[a][:, qq, :],
                                pt[:, a, qq * P:(qq + 1) * P],
                                v_sb[:, i, h, :],
                                start=(i == 0 and qq == 0),
                                stop=(i == 4 * j + qq),
                                skip_group_check=True,
                            )
                for a, tag in ((0, "rcA"), (1, "rcB")):
                    rc = evict.tile([P, NM], F32, tag=tag)
                    # ~18-bit reciprocal, 5x faster than exact
                    nc.vector.reciprocal_approx_fast(rc[:], yt[a][:, :, 0])
                    nc.vector.tensor_tensor(
                        y_sb[:, 4 * j:4 * j + NM, c, a, :],
                        yt[a][:, :, 1:],
                        rc[:, :, None].to_broadcast((P, NM, D)),
                        MULT)

            # ---- schedule: j-outer windows.  Per window: qk proj for all
            # pairs (unblocks the exp stream), then attention per pair with
            # deferred work (future v tiles, previous window's transposes +
            # out-proj) interleaved at pair boundaries.
            for c in range(GKC):
                qk_proj(c, 0)
                if c == 0:
                    v_tile(0)
                    v_tile(1)
            v_tile(2)
            v_tile(3)
            for j in range(NJ):
                if j > 0:
                    for c in range(GKC):
                        qk_proj(c, j)
                for c in range(GKC):
                    attention(c, j)
                    for qq in range(NM):
                        transpose_tile(c, 4 * j + qq)
                    if j + 1 < NJ:
                        v_tile(4 * (j + 1) + c)
                    if j > 0:
                        out_proj(4 * (j - 1) + c)
            for c in range(GKC):
                out_proj(4 * (NJ - 1) + c)

    # Bacc's compile pipeline splits multi-sem waits into event/nop
    # instructions, auto-inserts gpsimd library loads and ACT table loads,
    # and lowers extended insts.
    nc.compile()
    return nc


def make_masks(TQ=512, NM=4):
    f = np.arange(TQ)[None, :]
    p = np.arange(P)[:, None]
    m = np.stack([(f >= (P * k + p)) for k in range(NM)])
    return m.astype(ml_dtypes.bfloat16)


def make_in_maps(x, W_attn, b_attn, W_proj):
    bf16 = ml_dtypes.bfloat16
    e4 = ml_dtypes.float8_e4m3
    masks = make_masks(min(512, T), min(512, T) // P)
    idn = np.eye(P, dtype=np.float32).astype(bf16)
    xTs, x8s = [], []
    for b in range(B):
        xt = np.ascontiguousarray(np.asarray(x[b]).T).astype(bf16)
        xTs.append(xt)
        x8s.append(xt.astype(e4))
    per_g = []
    for g in range(2):
        s = slice(g * GC, (g + 1) * GC)
        wqk_bf = np.ascontiguousarray(np.concatenate(
            [W_attn[:, s], W_attn[:, C:][:, s]], axis=1)).astype(bf16)
        per_g.append({
            "wqk": wqk_bf.astype(e4),
            "bqk": np.concatenate([b_attn[s], b_attn[C:][s]]).astype(np.float32),
            "wv": np.ascontiguousarray(W_attn[:, 2 * C:][:, s]).astype(bf16),
            "bv": b_attn[2 * C:][s].astype(np.float32),
            "wp": np.ascontiguousarray(W_proj[s, :]).astype(bf16),
            "masks": masks,
            "idn": idn,
        })
    return [{"xT": xTs[core // 2], "x8": x8s[core // 2], **per_g[core % 2]}
            for core in range(8)]


_NC_CACHE = {}


def kernel(x, W_attn, b_attn, W_proj, b_proj):
    x = np.asarray(x)
    W_attn = np.asarray(W_attn)
    b_attn = np.asarray(b_attn)
    W_proj = np.asarray(W_proj)
    b_proj = np.asarray(b_proj)

    if "nc" not in _NC_CACHE:
        _NC_CACHE["nc"] = build_nc()
    nc = _NC_CACHE["nc"]
    in_maps = make_in_maps(x, W_attn, b_attn, W_proj)
    try:
        res = run_bass_kernel_spmd(nc, in_maps, list(range(8)), trace=False)
    except Exception:
        # transient NRT_EXEC_UNIT_UNRECOVERABLE device wedges have been
        # observed on this fleet; one retry usually clears them
        import time as _time
        _time.sleep(5)
        res = run_bass_kernel_spmd(nc, in_maps, list(range(8)), trace=False)
    out = np.empty((B, T, C), np.float32)
    for b in range(B):
        out[b] = res.results[2 * b]["out"] + res.results[2 * b + 1]["out"] \
            + b_proj[None, :]
    return out


# revision 4
# speedup vs baseline: 1.1645x; 1.1104x over previous
# Causal self-attention (B=4, T=2048, C=1024, 16 heads) on 8 NeuronCores.
#
# Sharding: core i = (batch b = i//2, head-group g = i%2).  Each core runs the
# full attention pipeline for one batch element and 8 of the 16 heads:
#   qT,kT = Wqk^T @ x^T   fp8e4 DoubleRow matmuls.  W is stored hi+lo (two
#                         fp8 residual terms, pre-scaled x32 so the residual
#                         clears the e4m3 subnormal floor) -> W is ~13-bit
#                         exact and only the x quantization remains.  The DVE
#                         eviction computes psum/32 + bias and quantizes.
#   v     = x @ Wv + bv   bf16 (fp8 on the v path costs too much accuracy)
#   S^T   = kT^T @ qT     fp8e4 DoubleRow, K=64 on 64 partitions; the second
#                         slot holds the k residual (k_lo) so k is ~13-bit
#                         exact; q is broadcast (stride-0) single-fp8.
#   P     = exp(S/8) * causal-mask          (ACT exp straight from PSUM, bf16)
#   y_q   = P-chunks^T @ v_aug              (stationary P [128k,128q], moving
#           v [128k,65] -> out [128q,65]: full-width output partitions; the 4
#           query-subtile accumulation groups share one PSUM bank via a
#           single-start lazy-zero pattern)
#   y     = y_q[:,1:65] * recip(y_q[:,0])   (denominator from v ones column)
#   yT    = PE-transpose(y)                 (query-major -> head-dim-major)
#   out  += yT^T @ Wproj_g  (partial over head-group; summed on host)
# Host adds the two per-batch partials + b_proj.
#
# Scheduling: the ACT exp stream is the longest engine program (~147us) and
# must never starve.  The attention i-loop is software-pipelined (scores for
# tile i+1 are emitted before the AV matmuls of tile i) and all other PE work
# (v proj, qk proj of later pairs, y transposes, out proj) is chopped into
# small closures injected one-per-tile into the PE idle slots of the i-loop.
import numpy as np
import ml_dtypes

import concourse.tile as tile
from concourse import bacc, mybir
from concourse.bass_utils import run_bass_kernel_spmd

BF16 = mybir.dt.bfloat16
F8 = mybir.dt.float8e4
F32 = mybir.dt.float32
AF = mybir.ActivationFunctionType
MULT = mybir.AluOpType.mult
ADD = mybir.AluOpType.add
DR = mybir.MatmulPerfMode.DoubleRow

# Full-problem constants (hardcoded; kernel.py must be self-contained).
B, T, C, N_HEAD = 4, 2048, 1024, 16
D = C // N_HEAD            # 64
H = N_HEAD // 2            # 8 heads per core
GC = H * D                 # 512 group cols
P = 128
WSCALE = 32.0              # Wqk pre-scale so the fp8 residual is representable


def build_nc(T=T, C=C, H=H, D=D, trace=False):
    """Build the single-core Bass/Tile program (shared SPMD across 8 cores)."""
    KC = C // P                 # contraction chunks for C
    NT = T // P                 # token tiles
    TQ = min(512, T)            # query-chunk width
    NJ = T // TQ                # query chunks (windows)
    NM = TQ // P                # query subtiles per chunk
    GC_ = H * D
    GKC = GC_ // P              # head pairs
    VW = D + 1                  # per-head v width incl. ones column

    nc = bacc.Bacc("TRN2", target_bir_lowering=False, debug=False)

    xT_d = nc.dram_tensor("xT", [C, T], BF16, kind="ExternalInput")
    x8_d = nc.dram_tensor("x8", [C, T], F8, kind="ExternalInput")
    wqk_d = nc.dram_tensor("wqk", [C, 2, 2 * GC_], F8, kind="ExternalInput")
    bqk_d = nc.dram_tensor("bqk", [2 * GC_], F32, kind="ExternalInput")
    wv_d = nc.dram_tensor("wv", [C, GC_], BF16, kind="ExternalInput")
    bv_d = nc.dram_tensor("bv", [GC_], F32, kind="ExternalInput")
    wp_d = nc.dram_tensor("wp", [GC_, C], BF16, kind="ExternalInput")
    masks_d = nc.dram_tensor("masks", [NM, P, TQ], BF16, kind="ExternalInput")
    idn_d = nc.dram_tensor("idn", [P, P], BF16, kind="ExternalInput")
    out_d = nc.dram_tensor("out", [T, C], F32, kind="ExternalOutput")

    scale = float(1.0 / np.sqrt(D))

    with tile.TileContext(nc) as tc:
        with (
            tc.tile_pool(name="persist", bufs=1) as persist,
            tc.tile_pool(name="ptile", bufs=12) as ptile,
            tc.tile_pool(name="evict", bufs=2) as evict,
            tc.tile_pool(name="s_psum", bufs=2, space="PSUM") as s_psum,
            tc.tile_pool(name="y_psum", bufs=1, space="PSUM") as y_psum,
            tc.tile_pool(name="mm_psum", bufs=2, space="PSUM") as mm_psum,
        ):
            # ---- persistent SBUF tensors.  DMA order = first-use order.
            bqk_sb = persist.tile([P, 2 * GKC], F32)
            nc.sync.dma_start(bqk_sb[:], bqk_d.ap().rearrange("(m p) -> p m", p=P))
            bv_sb = persist.tile([1, GC_], F32)
            nc.sync.dma_start(bv_sb[:], bv_d.ap()[None, :])
            idn_sb = persist.tile([P, P], BF16)
            nc.sync.dma_start(idn_sb[:], idn_d.ap())

            wqk_sb = persist.tile([P, KC, 2, 2 * GC_], F8)
            wqk_r = wqk_d.ap().rearrange("(kc p) two m -> p kc two m", p=P)
            nc.sync.dma_start(wqk_sb[:], wqk_r)

            x8_sb = persist.tile([P, KC, T], F8)
            x8_r = x8_d.ap().rearrange("(kc p) t -> p kc t", p=P)
            nc.sync.dma_start(x8_sb[:, :, 0:TQ], x8_r[:, :, 0:TQ])

            masks_sb = persist.tile([P, NM, TQ], BF16)
            nc.sync.dma_start(masks_sb[:], masks_d.ap().rearrange("m p f -> p m f"))

            wv_sb = persist.tile([P, KC, GC_], BF16)
            nc.sync.dma_start(wv_sb[:], wv_d.ap().rearrange("(kc p) m -> p kc m", p=P))
            xT_sb = persist.tile([P, KC, T], BF16)
            xT_r = xT_d.ap().rearrange("(kc p) t -> p kc t", p=P)
            for t in range(NM):  # first window's token tiles, fine-grained
                ts_ = slice(t * P, (t + 1) * P)
                nc.sync.dma_start(xT_sb[:, :, ts_], xT_r[:, :, ts_])

            js1 = slice(TQ, 2 * TQ)
            nc.sync.dma_start(x8_sb[:, :, js1], x8_r[:, :, js1])
            nc.sync.dma_start(xT_sb[:, :, js1], xT_r[:, :, js1])
            wp_sb = persist.tile([P, GKC, C], BF16)
            nc.sync.dma_start(wp_sb[:], wp_d.ap().rearrange("(kc p) m -> p kc m", p=P))
            for j in range(2, NJ):
                js = slice(j * TQ, (j + 1) * TQ)
                nc.sync.dma_start(x8_sb[:, :, js], x8_r[:, :, js])
                nc.sync.dma_start(xT_sb[:, :, js], xT_r[:, :, js])

            # gpsimd: bias broadcast for the v eviction
            bvb = persist.tile([P, GC_], F32)
            nc.gpsimd.partition_broadcast(bvb[:], bv_sb[:])

            # qk8 layout: [p, pair, {0:q, 1:k_hi, 2:k_lo}, T].  Head A of
            # pair c lives on partitions 0-63, head B on 64-127.
            qk8_sb = persist.tile([P, GKC, 3, T], F8)

            # DVE "touch": absorb DMA waits into the DVE vector clock before
            # their first 2-input consumers.
            scrap = persist.tile([P, 2], F32)
            nc.vector.tensor_copy(scrap[:, 0:1], bqk_sb[:, 0:1])
            nc.vector.tensor_copy(scrap[:, 1:2], masks_sb[:, 0, 0:1])

            v_sb = persist.tile([P, NT, H, VW], BF16)
            nc.vector.memset(v_sb[:, :, :, 0:1], 1.0)     # ones columns only
            y_sb = persist.tile([P, NT, GKC, 2, D], BF16)  # partition = query
            yT_sb = persist.tile([P, GKC, T], BF16)        # partition = (a,d)

            # ---- phase pieces --------------------------------------------
            def v_tile(t):
                """v = x @ Wv + bv for one 128-token tile (bf16)."""
                ps = mm_psum.tile([P, TQ], F32, tag="mm")
                for kc in range(KC):
                    nc.tensor.matmul(
                        ps[:],
                        xT_sb[:, kc, t * P:(t + 1) * P],
                        wv_sb[:, kc, :],
                        start=(kc == 0), stop=(kc == KC - 1),
                    )
                nc.vector.tensor_tensor(
                    v_sb[:, t, :, 1:],
                    ps[:].rearrange("p (h e) -> p h e", h=H),
                    bvb[:].rearrange("p (h e) -> p h e", h=H),
                    ADD,
                )

            def qk_proj(c, j):
                """q,k chunks for pair c, window j: fp8 DoubleRow with W in
                hi+lo residual form (W exact to ~13 bits, x single-fp8).
                Eviction computes psum/WSCALE + bias -> fp8; the k eviction
                additionally stores the k residual (gpsimd) for the score
                DoubleRow second slot."""
                js = slice(j * TQ, (j + 1) * TQ)
                for qk, m in ((0, c), (1, GKC + c)):
                    ps = mm_psum.tile([P, TQ], F32, tag="mm")
                    for kc in range(KC):
                        nc.tensor.matmul(
                            ps[:],
                            wqk_sb[:, kc, :, m * P:(m + 1) * P],
                            x8_sb[:, kc, None, js].to_broadcast((P, 2, TQ)),
                            start=(kc == 0), stop=(kc == KC - 1),
                            perf_mode=DR,
                        )
                    if qk == 0:
                        nc.vector.tensor_scalar(
                            qk8_sb[:, c, 0, js], ps[:],
                            1.0 / WSCALE, bqk_sb[:, m:m + 1], MULT, ADD)
                    else:
                        khb = evict.tile([P, TQ], F32, tag="khb", name="khb")
                        nc.vector.tensor_scalar(
                            khb[:], ps[:],
                            1.0 / WSCALE, bqk_sb[:, m:m + 1], MULT, ADD)
                        nc.vector.tensor_copy(qk8_sb[:, c, 1, js], khb[:])
                        nc.gpsimd.tensor_sub(
                            qk8_sb[:, c, 2, js], khb[:], qk8_sb[:, c, 1, js])

            def out_proj_half(t, nn, cell):
                """One 512-col half of out = yT^T @ Wproj for token tile t."""
                def f():
                    if "ot" not in cell:
                        cell["ot"] = evict.tile([P, C], F32, tag="out",
                                                bufs=3, name="ot")
                    ps = mm_psum.tile([P, TQ], F32, tag="mm")
                    for kc in range(GKC):
                        nc.tensor.matmul(
                            ps[:],
                            yT_sb[:, kc, t * P:(t + 1) * P],
                            wp_sb[:, kc, nn * TQ:(nn + 1) * TQ],
                            start=(kc == 0), stop=(kc == GKC - 1),
                        )
                    nc.vector.tensor_copy(
                        cell["ot"][:, nn * TQ:(nn + 1) * TQ], ps[:])
                    if nn == 1:
                        nc.sync.dma_start(
                            out_d.ap()[t * P:(t + 1) * P, :], cell["ot"])
                return f

            def transpose_tile(c, qt):
                """y_sb [128q, (a d)] -> yT_sb [(a d), 128q] via PE transpose
                through a borrowed mm-ring PSUM slot."""
                tp = mm_psum.tile([P, P], BF16, tag="mm", name="tp")
                nc.tensor.transpose(tp[:], y_sb[:, qt, c], idn_sb[:])
                nc.vector.tensor_copy(yT_sb[:, c, qt * P:(qt + 1) * P], tp[:])

            dq = []  # deferred PE work, injected into attention idle slots

            def pop_deferred():
                if dq:
                    dq.pop(0)()

            def attention(c, j):
                """Scores+exp+mask+AV+normalize for pair c, window j.
                Software-pipelined: scores for tile i+1 are emitted before the
                AV matmuls of tile i so the PE never gates the exp stream."""
                hA, hB = 2 * c, 2 * c + 1
                yt = {}
                for a, tag in ((0, "yA"), (1, "yB")):
                    yt[a] = y_psum.tile([P, NM, VW], F32, tag=tag, name=tag)
                ilast = 4 * j + NM - 1
                pts = {}

                def emit_scores(i):
                    m = i - 4 * j
                    lo = P * m if m > 0 else 0
                    ls = slice(lo, TQ)
                    cs = slice(j * TQ + lo, (j + 1) * TQ)
                    kt = slice(i * P, (i + 1) * P)
                    st = s_psum.tile([P, 2, TQ], F32, tag="s", name="st")
                    for a in (0, 1):
                        pb = slice(64 * a, 64 * a + 64)
                        nc.tensor.matmul(
                            st[:, a, ls],
                            qk8_sb[pb, c, 1:3, kt],
                            qk8_sb[pb, c, 0:1, cs].to_broadcast(
                                (64, 2, TQ - lo)),
                            start=True, stop=True, perf_mode=DR,
                        )
                    pt = ptile.tile([P, 2, TQ], BF16, tag="p", name="pt")
                    nc.scalar.activation(pt[:, :, ls], st[:, :, ls],
                                         AF.Exp, scale=scale)
                    if m >= 0:  # diagonal: causal mask (same for A and B)
                        with tc.high_priority():
                            nc.vector.tensor_tensor(
                                pt[:, :, ls], pt[:, :, ls],
                                masks_sb[:, m, None, ls].to_broadcast(
                                    (P, 2, TQ - lo)),
                                MULT)
                    pts[i] = pt

                emit_scores(0)
                for i in range(ilast + 1):
                    if i + 1 <= ilast:
                        emit_scores(i + 1)
                    pop_deferred()
                    m = i - 4 * j
                    q0 = max(m, 0)
                    pt = pts.pop(i)
                    for a, h in ((0, hA), (1, hB)):
                        for qq in range(q0, NM):
                            nc.tensor.matmul(
                                yt[a][:, qq, :],
                                pt[:, a, qq * P:(qq + 1) * P],
                                v_sb[:, i, h, :],
                                start=(i == 0 and qq == 0),
                                stop=(i == 4 * j + qq),
                                skip_group_check=True,
                            )
                for a, tag in ((0, "rcA"), (1, "rcB")):
                    rc = evict.tile([P, NM], F32, tag=tag, name=tag)
                    # ~18-bit reciprocal, 5x faster than exact
                    nc.vector.reciprocal_approx_fast(rc[:], yt[a][:, :, 0])
                    nc.vector.tensor_tensor(
                        y_sb[:, 4 * j:4 * j + NM, c, a, :],
                        yt[a][:, :, 1:],
                        rc[:, :, None].to_broadcast((P, NM, D)),
                        MULT)

            # ---- schedule ------------------------------------------------
            def mk(f, *args):
                return lambda: f(*args)

            for c in range(GKC):
                qk_proj(c, 0)
            for j in range(NJ):
                if j > 0:
                    qk_proj(0, j)
                    dq[0:0] = ([mk(qk_proj, c, j) for c in (1, 2, 3)]
                               + [mk(v_tile, 4 * j + k) for k in range(NM)
                                  if j < NJ])
                else:
                    dq[0:0] = [mk(v_tile, k) for k in range(NM)]
                for c in range(GKC):
                    attention(c, j)
                    dq.extend(mk(transpose_tile, c, 4 * j + qq)
                              for qq in range(NM))
                    if c == 1 and j >= 1:
                        for t in range(4 * (j - 1), 4 * j):
                            cell = {}
                            dq.append(out_proj_half(t, 0, cell))
                            dq.append(out_proj_half(t, 1, cell))
            while dq:
                dq.pop(0)()
            for t in range(4 * (NJ - 1), NT):
                cell = {}
                out_proj_half(t, 0, cell)()
                out_proj_half(t, 1, cell)()

    nc.compile()
    return nc


def make_masks(TQ=512, NM=4):
    f = np.arange(TQ)[None, :]
    p = np.arange(P)[:, None]
    m = np.stack([(f >= (P * k + p)) for k in range(NM)])
    return m.astype(ml_dtypes.bfloat16)


def make_in_maps(x, W_attn, b_attn, W_proj):
    bf16 = ml_dtypes.bfloat16
    e4 = ml_dtypes.float8_e4m3
    masks = make_masks(min(512, T), min(512, T) // P)
    idn = np.eye(P, dtype=np.float32).astype(bf16)
    xTs, x8s = [], []
    for b in range(B):
        xt = np.ascontiguousarray(np.asarray(x[b]).T).astype(bf16)
        xTs.append(xt)
        x8s.append(xt.astype(e4))
    per_g = []
    for g in range(2):
        s = slice(g * GC, (g + 1) * GC)
        wqk_bf = np.ascontiguousarray(np.concatenate(
            [W_attn[:, s], W_attn[:, C:][:, s]], axis=1)).astype(bf16)
        ws = wqk_bf.astype(np.float32) * WSCALE
        whi = ws.astype(e4)
        wlo = (ws - whi.astype(np.float32)).astype(e4)
        per_g.append({
            "wqk": np.ascontiguousarray(np.stack([whi, wlo], axis=1)),
            "bqk": np.concatenate([b_attn[s], b_attn[C:][s]]).astype(np.float32),
            "wv": np.ascontiguousarray(W_attn[:, 2 * C:][:, s]).astype(bf16),
            "bv": b_attn[2 * C:][s].astype(np.float32),
            "wp": np.ascontiguousarray(W_proj[s, :]).astype(bf16),
            "masks": masks,
            "idn": idn,
        })
    return [{"xT": xTs[core // 2], "x8": x8s[core // 2], **per_g[core % 2]}
            for core in range(8)]


_NC_CACHE = {}


def kernel(x, W_attn, b_attn, W_proj, b_proj):
    x = np.asarray(x)
    W_attn = np.asarray(W_attn)
    b_attn = np.asarray(b_attn)
    W_proj = np.asarray(W_proj)
    b_proj = np.asarray(b_proj)

    if "nc" not in _NC_CACHE:
        _NC_CACHE["nc"] = build_nc()
    nc = _NC_CACHE["nc"]
    in_maps = make_in_maps(x, W_attn, b_attn, W_proj)
    try:
        res = run_bass_kernel_spmd(nc, in_maps, list(range(8)), trace=False)
    except Exception:
        # transient NRT_EXEC_UNIT_UNRECOVERABLE device wedges have been
        # observed on this fleet; one retry usually clears them
        import time as _time
        _time.sleep(5)
        res = run_bass_kernel_spmd(nc, in_maps, list(range(8)), trace=False)
    out = np.empty((B, T, C), np.float32)
    for b in range(B):
        out[b] = res.results[2 * b]["out"] + res.results[2 * b + 1]["out"] \
            + b_proj[None, :]
    return out
